# revision 23
# baseline (speedup 1.0000x reference)
"""3-layer GAT (GATConv x3 + log_softmax) on 8 trn2 NeuronCores — v3.

Changes vs the v2 baseline:
- Layer-1 node phase is REPLICATED: every core computes h1|als1 for ALL
  nodes straight from the full x input (Fin=12, cheap on PE) and writes
  HF1 locally -> the 64MB layer-1 AllGather is gone entirely.  al_d for
  own windows comes from a tiny extra matmul against the own-shard
  input (xTo), since "own windows" is not static under SPMD.
- h rows are fp8(e4m3) 768B [512 fp8 h | 8 bf16 als | pad] instead of
  bf16 1280B: gather reads, HF writes and the boundary AllGathers all
  shrink 40%.  al_s rides in the row as raw bf16 bytes (bitcast view);
  al_d stays bf16 (256B rows, separate small gather).
- The h row space is split into 3 chunks (21/21/7 windows) with one
  Shared tensor per chunk, so each boundary AllGather fires as soon as
  its windows close (w=20/41/48) and overlaps the edge phase.  One
  unified id/table set serves all three layers.
- Node-phase HF1 stores are batched 7 windows per DMA (strided 3D out
  AP) - per-window DMAs serialized the SP queue.
- Engine balancing: PSUM->SBUF copies and exp-expansion alternate
  between ACT and DVE.
"""
import numpy as np
import ml_dtypes

import concourse.bass as bass
import concourse.mybir as mybir
import concourse.tile as tile
from concourse.bass_utils import run_bass_kernel_spmd

BF = ml_dtypes.bfloat16
N = 50000
NC = 8
SHARD = N // NC            # 6250
W = (SHARD + 127) // 128   # 49 windows of 128 dst nodes
SHARD_PAD = W * 128        # 6272
H, C = 8, 64
F = H * C                  # 512
C3 = 5
F3 = H * C3                # 40
EW = 640                   # (unused for h rows now; kept for reference)
EW8 = 768                  # fp8 h row width: 512 h fp8 | 16B als bf16 | pad
HB = 528                   # written bytes per h row
EW3 = 64                   # layer-3 row width (f32): 40 h | 8 als | pad
CHB = (0, 7, 28, 49)       # chunk window boundaries (3 AllGather chunks)
NQ = 3
QNW = tuple(CHB[q + 1] - CHB[q] for q in range(NQ))    # (21, 21, 7)
QROWS = tuple(NC * nw * 128 for nw in QNW)             # per-chunk rows
NEG_SLOPE = 0.2
DT_BF = mybir.dt.bfloat16
DT_F32 = mybir.dt.float32
DT_I16 = mybir.dt.int16
DT_F8 = mybir.dt.float8e4
AF = mybir.ActivationFunctionType
ALU = mybir.AluOpType


PAIRS = [(2 * p, 2 * p + 1) for p in range(W // 2)] + [(W - 1,)]


def _split_drain_waits(nc, max_waits=1):
    # walrus on this toolchain rejects instructions carrying more than a few
    # sync waits; keep <=max_waits per instruction, move extras onto NoOps
    # inserted just before (same engine -> executes first, semantics kept).
    ctr = 0
    for f in nc.m.functions:
        for blk in f.blocks:
            new_list = []
            for ins in blk.instructions:
                if ins.sync_info and \
                        len(ins.sync_info.on_wait) > max_waits:
                    waits = list(ins.sync_info.on_wait)
                    keep, extra = waits[:max_waits], waits[max_waits:]
                    for w in extra:
                        ctr += 1
                        new_list.append(mybir.InstNoOp(
                            name=f"drainfix-{ctr}", engine=ins.engine,
                            ins=[], outs=[],
                            sync_info=mybir.SyncInfo(on_wait=[w], on_update=[])))
                    ins.sync_info.on_wait = keep
                new_list.append(ins)
            blk.instructions[:] = new_list


def _bcast(ap, ap_list):
    return bass.AP(ap.tensor, ap.offset, ap_list)


def blockdiag(a):
    Hh, cc = a.shape
    out = np.zeros((Hh * cc, Hh), np.float32)
    for h in range(Hh):
        out[h * cc:(h + 1) * cc, h] = a[h]
    return out


def _rowq(g):
    # unified chunked row space: (chunk q, row within chunk tensor)
    c, r = np.divmod(g, SHARD)
    w, d = np.divmod(r, 128)
    q = np.searchsorted(np.asarray(CHB), w, side="right") - 1
    base = np.asarray(CHB)[q]
    nw = np.asarray(QNW)[q]
    return q, c * nw * 128 + (w - base) * 128 + d


def _space_tables(src, dst):
    """Per-core gather tables for the unified chunked row space."""
    q_all, rows_all = _rowq(src)

    edges = [[[None] * W for _ in range(NC)] for _ in range(NQ)]
    for c in range(NC):
        lo, hi = c * SHARD, (c + 1) * SHARD
        m0 = np.searchsorted(dst, lo, "left")
        m1 = np.searchsorted(dst, hi, "left")
        d_c = dst[m0:m1] - lo
        rq = rows_all[m0:m1]
        qq = q_all[m0:m1]
        counts = np.bincount(d_c, minlength=SHARD)
        starts = np.concatenate([[0], np.cumsum(counts)])
        for w in range(W):
            n0, n1 = w * 128, min((w + 1) * 128, SHARD)
            e0, e1 = starts[n0], starts[n1]
            dw = d_c[e0:e1] - w * 128
            for q in range(NQ):
                mq = qq[e0:e1] == q
                edges[q][c][w] = (rq[e0:e1][mq].astype(np.int32), dw[mq])

    KQ = [[max(max(-(-len(edges[q][c][w][0]) // 128) for c in range(NC)), 1)
           for w in range(W)] for q in range(NQ)]

    pair_meta = []      # (ws, kq (NQ x len(ws)), colq (NQ), tile0)
    colq = [0] * NQ
    cT = 0
    for ws in PAIRS:
        kq = [[KQ[q][w] for w in ws] for q in range(NQ)]
        pair_meta.append((ws, kq, tuple(colq), cT))
        for q in range(NQ):
            colq[q] += sum(kq[q])
        cT += sum(sum(k) for k in kq)

    CQ = [colq[q] * 8 for q in range(NQ)]
    CT = cT

    def wrap16(flat, tbl, col0):
        a = flat.reshape(-1, 16)
        for g in range(8):
            tbl[16 * g:16 * g + 16, col0:col0 + a.shape[0]] = a.T

    idxQ = [np.zeros((NC, 128, CQ[q]), np.int16) for q in range(NQ)]
    idxD = np.zeros((NC, 128, CT * 8), np.int16)
    drow = np.full((NC, 128, CT), 999.0, np.float32)

    for c in range(NC):
        for (ws, kq, q0s, t0) in pair_meta:
            ntq = [sum(kq[q]) for q in range(NQ)]
            ktp = sum(ntq)
            flatD = np.zeros(ktp * 128, np.int16)
            flatR = np.full(ktp * 128, 999.0, np.float32)
            tb = 0
            for q in range(NQ):
                flatQ = np.zeros(ntq[q] * 128, np.int16)
                qb = 0
                for wi, w in enumerate(ws):
                    s, d = edges[q][c][w]
                    n = len(s)
                    flatQ[qb * 128:qb * 128 + n] = s.astype(np.int16)
                    flatD[(tb + qb) * 128:(tb + qb) * 128 + n] = \
                        (d + w * 128).astype(np.int16)
                    flatR[(tb + qb) * 128:(tb + qb) * 128 + n] = \
                        d.astype(np.float32)
                    qb += kq[q][wi]
                wrap16(flatQ, idxQ[q][c], q0s[q] * 8)
                tb += ntq[q]
            wrap16(flatD, idxD[c], t0 * 8)
            drow[c, :, t0:t0 + ktp] = flatR.reshape(-1, 128).T

    return dict(idxQ=idxQ, idxD=idxD, drow=drow,
                pair_meta=pair_meta, CQ=CQ, CT=CT,
                KQ=tuple(tuple(k) for k in KQ))


def host_prep(edge_index):
    src = np.concatenate([edge_index[0], np.arange(N, dtype=np.int32)])
    dst = np.concatenate([edge_index[1], np.arange(N, dtype=np.int32)])
    order = np.argsort(dst, kind="stable")
    src, dst = src[order], dst[order]
    ts = _space_tables(src, dst)
    meta = ts["KQ"]
    return meta, {"ts": ts}


def build_program(meta, tables, null=False, debug_stage=99):
    use_bias = tables.get("use_bias", True)
    ts = tables["ts"]

    nc = bass.Bass("TRN2")
    P = {}

    def par(name, shape, dt):
        P[name] = nc.declare_dram_parameter(name, list(shape), dt, isOutput=False)
        return P[name]

    par("xT", [12, NC * SHARD_PAD], DT_BF)
    par("xTo", [12, SHARD_PAD], DT_BF)
    par("w1", [12, F], DT_BF)
    par("wwa1", [12, 16], DT_BF)
    par("w2c", [4, 128, F], DT_BF)
    par("wwa2", [4, 128, 16], DT_BF)
    par("w3c", [4, 128, F3], DT_BF)
    par("wwa3", [4, 128, 16], DT_BF)
    par("b1r", [128, F], DT_F32)
    par("b2r", [128, F], DT_F32)
    par("b3r", [128, C3], DT_F32)
    par("iotab", [128, 128], DT_BF)
    par("identb", [128, 128], DT_BF)
    for q in range(NQ):
        par(f"idxQ{q}", [128, ts["CQ"][q]], DT_I16)
    par("idxD", [128, ts["CT"] * 8], DT_I16)
    par("drow", [128, ts["CT"]], DT_F32)
    par("tick", [128, 1], DT_F32)
    OUT = nc.declare_dram_parameter("out", [SHARD, C3], DT_F32, isOutput=True)
    TOCK = nc.declare_dram_parameter("tock", [128, 1], DT_F32, isOutput=True)

    if null:
        with tile.TileContext(nc) as tc:
            with tc.tile_pool(name="s", bufs=1) as s0:
                z = s0.tile([128, C3], DT_F32)
                nc.vector.memset(z[:], 0.0)
                for r0 in range(0, SHARD, 128):
                    nc.sync.dma_start(out=OUT[r0:min(r0 + 128, SHARD), :],
                                      in_=z[:min(128, SHARD - r0), :])
                tk = s0.tile([128, 1], DT_F32)
                nc.sync.dma_start(out=tk[:], in_=P["tick"][:])
                nc.sync.dma_start(out=TOCK[:], in_=tk[:])
        _finalize(nc)
        return nc

    rg = [list(range(NC))]
    with tile.TileContext(nc) as tc:
        with (
            tc.tile_pool(name="const", bufs=1) as cp,
            tc.tile_pool(name="sbuf", bufs=3) as sb,
            tc.tile_pool(name="stage", bufs=2) as stg,
            tc.tile_pool(name="selp", bufs=3) as selp,
            tc.tile_pool(name="xstg", bufs=1) as xstg,
            tc.tile_pool(name="gath", bufs=3) as gth,
            tc.tile_pool(name="psU", bufs=3, space="PSUM") as p_U,
            tc.tile_pool(name="psX", bufs=2, space="PSUM") as p_X,
            tc.tile_pool(name="psT", bufs=2, space="PSUM") as p_T,
            tc.tile_pool(name="dram", bufs=1, space="DRAM") as dr,
        ):
            # ---------------- constants ----------------
            t_iota = cp.tile([128, 128], DT_BF)
            nc.sync.dma_start(out=t_iota[:], in_=P["iotab"][:])
            t_ident = cp.tile([128, 128], DT_BF)
            nc.sync.dma_start(out=t_ident[:], in_=P["identb"][:])
            t_xTo = cp.tile([12, SHARD_PAD], DT_BF)
            nc.sync.dma_start(out=t_xTo[:], in_=P["xTo"][:])
            t_w1 = cp.tile([12, F], DT_BF)
            nc.sync.dma_start(out=t_w1[:], in_=P["w1"][:])
            t_wwa1 = cp.tile([12, 16], DT_BF)
            nc.sync.dma_start(out=t_wwa1[:], in_=P["wwa1"][:])
            t_w2 = cp.tile([128, 4, F], DT_BF)
            t_wwa2 = cp.tile([128, 4, 16], DT_BF)
            t_w3 = cp.tile([128, 4, F3], DT_BF)
            t_wwa3 = cp.tile([128, 4, 16], DT_BF)
            for ch in range(4):
                nc.sync.dma_start(out=t_w2[:, ch, :], in_=P["w2c"][ch])
                nc.sync.dma_start(out=t_wwa2[:, ch, :], in_=P["wwa2"][ch])
                nc.sync.dma_start(out=t_w3[:, ch, :], in_=P["w3c"][ch])
                nc.sync.dma_start(out=t_wwa3[:, ch, :], in_=P["wwa3"][ch])
            if use_bias:
                t_b1 = cp.tile([128, F], DT_F32)
                nc.sync.dma_start(out=t_b1[:], in_=P["b1r"][:])
                t_b2 = cp.tile([128, F], DT_F32)
                nc.sync.dma_start(out=t_b2[:], in_=P["b2r"][:])
            t_b3 = cp.tile([128, C3], DT_F32)
            if use_bias:
                nc.sync.dma_start(out=t_b3[:], in_=P["b3r"][:])
            t_idxQ = []
            for q in range(NQ):
                tq = cp.tile([128, ts["CQ"][q]], DT_I16, name=f"t_idxQ{q}")
                nc.sync.dma_start(out=tq[:], in_=P[f"idxQ{q}"][:])
                t_idxQ.append(tq)
            t_idxD = cp.tile([128, ts["CT"] * 8], DT_I16)
            nc.sync.dma_start(out=t_idxD[:], in_=P["idxD"][:])
            t_drow = cp.tile([128, ts["CT"]], DT_F32)
            nc.sync.dma_start(out=t_drow[:], in_=P["drow"][:])

            # ---------------- DRAM internals ----------------
            HF1 = [dr.tile([QROWS[q], EW8], DT_F8, name=f"HF1_{q}")
                   for q in range(NQ)]
            AD1 = dr.tile([SHARD_PAD, 128], DT_BF)
            AD2 = dr.tile([SHARD_PAD, 128], DT_BF)
            AD3 = dr.tile([SHARD_PAD, EW3], DT_F32)
            EX2 = [dr.tile([QNW[q] * 128, EW8], DT_F8, name=f"EX2_{q}")
                   for q in range(NQ)]
            HF2 = [dr.tile([QROWS[q], EW8], DT_F8, addr_space="Shared",
                           name=f"HF2_{q}") for q in range(NQ)]
            EX3 = [dr.tile([QNW[q] * 128, EW3], DT_F32, name=f"EX3_{q}")
                   for q in range(NQ)]
            H3F = [dr.tile([QROWS[q], EW3], DT_F32, addr_space="Shared",
                           name=f"H3F_{q}") for q in range(NQ)]
            OUTI = dr.tile([SHARD_PAD, C3], DT_F32)

            # ---------------- layer-1 node phase (replicated) ------------
            # al_d for own windows, from own-shard input
            for w in range(W):
                pao = p_X.tile([128, 24], DT_F32, space="PSUM", tag="paux")
                nc.tensor.matmul(pao[:, 0:8], lhsT=t_xTo[:, w * 128:(w + 1) * 128],
                                 rhs=t_wwa1[:, 8:16], start=True, stop=True)
                ast = stg.tile([128, 8], DT_BF, tag="ast")
                nc.vector.tensor_copy(out=ast[:], in_=pao[:, 0:8])
                nc.sync.dma_start(out=AD1[w * 128:(w + 1) * 128, 0:8],
                                  in_=ast[:])
            # h1 | als1 for ALL nodes; batched 7-window DMA writes
            for blk in range(NC):
                xch = xstg.tile([12, SHARD_PAD], DT_BF, tag="xch")
                nc.sync.dma_start(
                    out=xch[:],
                    in_=P["xT"][:, blk * SHARD_PAD:(blk + 1) * SHARD_PAD])
                for w7 in range(W // 7):
                    hst = stg.tile([128, 7, HB], DT_F8, tag="hst")
                    for wj in range(7):
                        w = w7 * 7 + wj
                        lhs = xch[:, w * 128:(w + 1) * 128]
                        ph = p_U.tile([128, F], DT_F32, space="PSUM",
                                      tag="pout")
                        nc.tensor.matmul(ph[:], lhsT=lhs, rhs=t_w1[:],
                                         start=True, stop=True)
                        pa = p_X.tile([128, 24], DT_F32, space="PSUM",
                                      tag="paux")
                        nc.tensor.matmul(pa[:, 0:8], lhsT=lhs,
                                         rhs=t_wwa1[:, 0:8],
                                         start=True, stop=True)
                        if (blk + w) % 2 == 0:
                            nc.scalar.activation(hst[:, wj, 0:F], ph[:],
                                                 AF.Copy)
                        else:
                            nc.vector.tensor_copy(out=hst[:, wj, 0:F],
                                                  in_=ph[:])
                        nc.vector.tensor_copy(
                            out=hst[:, wj, F:F + 16].bitcast(DT_BF),
                            in_=pa[:, 0:8])
                    w0 = w7 * 7
                    q = 0 if w0 < CHB[1] else (1 if w0 < CHB[2] else 2)
                    hf = HF1[q]
                    r0 = blk * QNW[q] * 128 + (w0 - CHB[q]) * 128
                    out_ap = bass.AP(
                        hf[:].tensor, (r0 * EW8) + hf[:].offset,
                        [[EW8, 128], [128 * EW8, 7], [1, HB]])
                    nc.sync.dma_start(out=out_ap, in_=hst[:])

            # ---------------- edge phases ----------------
            _regs = {}

            def nreg(v):
                if v not in _regs:
                    _regs[v] = nc.gpsimd.to_reg(v)
                return _regs[v]

            def edge_phase(layer):
                lay3 = layer == 3
                ADt = (AD1, AD2, AD3)[layer - 1]
                ald_ew = EW3 if lay3 else 128
                srcQ = (HF1, HF2, H3F)[layer - 1]
                ew = EW3 if lay3 else EW8
                hdt = DT_F32 if lay3 else DT_F8
                fh = F3 if lay3 else F
                cw = C3 if lay3 else C
                gdt = DT_F32 if lay3 else DT_BF
                als_off = F3 if lay3 else F
                for pi, (ws, kq, q0s, t0) in enumerate(ts["pair_meta"]):
                    ntq = [sum(kq[q]) for q in range(NQ)]
                    ktp = sum(ntq)
                    hg = gth.tile([128, ktp, ew], hdt, tag="hg")
                    tb = 0
                    for q in range(NQ):
                        nc.gpsimd.dma_gather(
                            hg[:, tb:tb + ntq[q], :], srcQ[q][:],
                            t_idxQ[q][:, q0s[q] * 8:(q0s[q] + ntq[q]) * 8],
                            ntq[q] * 128, nreg(ntq[q] * 128), ew,
                            single_packet=False)
                        tb += ntq[q]
                    ald = gth.tile([128, ktp, ald_ew], gdt, tag="ald")
                    nc.gpsimd.dma_gather(
                        ald[:], ADt[:], t_idxD[:, t0 * 8:(t0 + ktp) * 8],
                        ktp * 128, nreg(ktp * 128), ald_ew,
                        single_packet=False)
                    sels = []
                    for wi, w in enumerate(ws):
                        regions = []
                        tb = 0
                        for q in range(NQ):
                            regions.append((tb + sum(kq[q][:wi]), kq[q][wi]))
                            tb += ntq[q]
                        rsels = []
                        for (rb, rl) in regions:
                            sel = selp.tile([128, rl, 128], DT_BF, tag="sel")
                            for j in range(rl):
                                nc.vector.tensor_scalar(
                                    out=sel[:, j, :], in0=t_iota[:],
                                    scalar1=t_drow[:, t0 + rb + j:t0 + rb + j + 1],
                                    scalar2=None, op0=ALU.is_equal)
                            rsels.append((rb, rl, sel))
                        sels.append(rsels)
                    # e = als[src] + ald[dst]; ex = exp(lrelu(e))
                    ald_off = F3 + 8 if lay3 else 0
                    e_t = sb.tile([128, ktp, 8], gdt, tag="e")
                    if lay3:
                        als_v = hg[:, :, als_off:als_off + 8]
                    else:
                        als_v = hg[:, :, F:F + 16].bitcast(DT_BF)
                    nc.vector.tensor_tensor(
                        out=e_t[:], in0=als_v,
                        in1=ald[:, :, ald_off:ald_off + 8], op=ALU.add)
                    lr = sb.tile([128, ktp, 8], gdt, tag="lr")
                    nc.vector.scalar_tensor_tensor(
                        out=lr[:], in0=e_t[:], scalar=NEG_SLOPE,
                        in1=e_t[:], op0=ALU.mult, op1=ALU.max)
                    exb = sb.tile([128, ktp, 8], DT_BF, tag="exb")
                    nc.scalar.activation(exb[:], lr[:], AF.Exp)
                    # scatter per window
                    for wi, w in enumerate(ws):
                        poutF = p_U.tile([128, F], DT_F32, space="PSUM",
                                         tag="pout")
                        pout = poutF[:, 0:fh]
                        paux = p_X.tile([128, 24], DT_F32, space="PSUM",
                                        tag="paux")
                        pden = paux[:, 0:8]
                        nt_w = sum(kq[q][wi] for q in range(NQ))
                        i = 0
                        for (rb, rl, sel) in sels[wi]:
                            exw = sb.tile([128, rl, 8, cw],
                                          DT_F32 if lay3 else DT_BF,
                                          tag="exw")
                            exb_r = exb[:, rb:rb + rl, :]
                            exb_b = _bcast(exb_r, [exb_r.ap[0], [8, rl],
                                                   [1, 8], [0, cw]])
                            nc.scalar.activation(exw[:], exb_b, AF.Copy)
                            msg = sb.tile([128, rl, fh], DT_BF, tag="msg")
                            exw_f = _bcast(exw[:], [exw[:].ap[0], [fh, rl],
                                                    [1, fh]])
                            nc.vector.tensor_tensor(
                                out=msg[:], in0=hg[:, rb:rb + rl, 0:fh],
                                in1=exw_f, op=ALU.mult)
                            for j in range(rl):
                                st, sp_ = i == 0, i == nt_w - 1
                                nc.tensor.matmul(pout[:], lhsT=sel[:, j, :],
                                                 rhs=msg[:, j, :],
                                                 start=st, stop=sp_)
                                nc.tensor.matmul(pden[:], lhsT=sel[:, j, :],
                                                 rhs=exb[:, rb + j, :],
                                                 start=st, stop=sp_)
                                i += 1
                        _close(layer, w, pout, pden, paux)

            def ag(layer, q):
                exh = EX2[q] if layer == 1 else EX3[q]
                hfh = HF2[q] if layer == 1 else H3F[q]
                nc.gpsimd.collective_compute(
                    "AllGather", ALU.bypass, replica_groups=rg,
                    ins=[exh[:].opt()], outs=[hfh[:].opt()])

            def _close(layer, w, pout, pden, paux):
                lay3 = layer == 3
                fh = F3 if lay3 else F
                cw = C3 if lay3 else C
                r0 = w * 128
                den = sb.tile([128, 8], DT_F32, tag="den")
                nc.vector.tensor_scalar_add(den[:], pden[:], 1e-16)
                rec = sb.tile([128, 8], DT_F32, tag="rec")
                nc.vector.reciprocal(rec[:], den[:])
                onrm = sb.tile([128, fh], DT_F32, tag="onrm")
                rec_b = _bcast(rec[:], [rec[:].ap[0], [1, 8], [0, cw]])
                po4 = _bcast(pout[:], [pout[:].ap[0], [cw, 8], [1, cw]])
                on4 = _bcast(onrm[:], [onrm[:].ap[0], [cw, 8], [1, cw]])
                nc.vector.tensor_tensor(out=on4, in0=po4, in1=rec_b, op=ALU.mult)
                if lay3:
                    hm = sb.tile([128, C3], DT_F32, tag="hm")
                    on_T = _bcast(onrm[:], [onrm[:].ap[0], [1, C3], [C3, 8]])
                    nc.vector.reduce_sum(hm[:], on_T, axis=mybir.AxisListType.X)
                    nc.vector.tensor_scalar_mul(hm[:], hm[:], 0.125)
                    if use_bias:
                        nc.vector.tensor_add(out=hm[:], in0=hm[:], in1=t_b3[:])
                    mx = sb.tile([128, 1], DT_F32, tag="mx")
                    nc.vector.reduce_max(mx[:], hm[:], axis=mybir.AxisListType.X)
                    xc = sb.tile([128, C3], DT_F32, tag="xc")
                    nc.vector.tensor_tensor(out=xc[:], in0=hm[:],
                                            in1=mx[:].to_broadcast([128, C3]),
                                            op=ALU.subtract)
                    e5 = sb.tile([128, C3], DT_F32, tag="e5")
                    nc.scalar.activation(e5[:], xc[:], AF.Exp)
                    s5 = sb.tile([128, 1], DT_F32, tag="s5")
                    nc.vector.reduce_sum(s5[:], e5[:], axis=mybir.AxisListType.X)
                    lg = sb.tile([128, 1], DT_F32, tag="lg")
                    nc.scalar.activation(lg[:], s5[:], AF.Ln)
                    res = sb.tile([128, C3], DT_F32, tag="res")
                    nc.vector.tensor_tensor(out=res[:], in0=xc[:],
                                            in1=lg[:].to_broadcast([128, C3]),
                                            op=ALU.subtract)
                    nc.sync.dma_start(out=OUTI[r0:r0 + 128, :], in_=res[:])
                    return
                if use_bias:
                    xb = sb.tile([128, F], DT_F32, tag="xb")
                    nc.vector.tensor_add(out=xb[:], in0=onrm[:],
                                         in1=t_b1[:] if layer == 1 else t_b2[:])
                else:
                    xb = onrm
                xn = sb.tile([128, F], DT_BF, tag="xn")
                nc.scalar.activation(xn[:], xb[:], AF.Relu)
                xnT = sb.tile([128, 4, 128], DT_BF, tag="xnT")
                for ch in range(4):
                    ptx = p_T.tile([128, 128], DT_BF, space="PSUM", tag="ptxT")
                    nc.tensor.transpose(ptx[:], xn[:, ch * 128:(ch + 1) * 128],
                                        t_ident[:])
                    if ch % 2 == 0:
                        nc.vector.tensor_copy(out=xnT[:, ch, :], in_=ptx[:])
                    else:
                        nc.scalar.activation(xnT[:, ch, :], ptx[:], AF.Copy)
                wN = t_w2 if layer == 1 else t_w3
                wwaN = t_wwa2 if layer == 1 else t_wwa3
                fn = F if layer == 1 else F3
                phF = p_U.tile([128, F], DT_F32, space="PSUM", tag="pout")
                ph = phF[:, 0:fn]
                pa = paux[:, 8:24]
                for ch in range(4):
                    nc.tensor.matmul(ph[:], lhsT=xnT[:, ch, :], rhs=wN[:, ch, :],
                                     start=(ch == 0), stop=(ch == 3))
                    nc.tensor.matmul(pa[:], lhsT=xnT[:, ch, :], rhs=wwaN[:, ch, :],
                                     start=(ch == 0), stop=(ch == 3))
                qw = 0 if w < CHB[1] else (1 if w < CHB[2] else 2)
                hr0 = (w - CHB[qw]) * 128
                r0w = w * 128
                if layer == 1:
                    ast = stg.tile([128, 8], DT_BF, tag="ast")
                    nc.vector.tensor_copy(out=ast[:], in_=pa[:, 8:16])
                    nc.sync.dma_start(out=AD2[r0w:r0w + 128, 0:8], in_=ast[:])
                else:
                    as3 = stg.tile([128, 8], DT_F32, tag="as3")
                    nc.vector.tensor_copy(out=as3[:], in_=pa[:, 8:16])
                    nc.sync.dma_start(
                        out=AD3[r0w:r0w + 128, F3 + 8:F3 + 16], in_=as3[:])
                if layer == 1:
                    hst = stg.tile([128, HB], DT_F8, tag="hstc")
                    if w % 2 == 0:
                        nc.scalar.activation(hst[:, 0:F], ph[:], AF.Copy)
                    else:
                        nc.vector.tensor_copy(out=hst[:, 0:F], in_=ph[:])
                    nc.vector.tensor_copy(
                        out=hst[:, F:F + 16].bitcast(DT_BF), in_=pa[:, 0:8])
                    nc.sync.dma_start(
                        out=EX2[qw][hr0:hr0 + 128, 0:HB], in_=hst[:])
                else:
                    h3 = stg.tile([128, F3 + 8], DT_F32, tag="h3")
                    nc.vector.tensor_copy(out=h3[:, 0:F3], in_=ph[:])
                    nc.vector.tensor_copy(out=h3[:, F3:F3 + 8], in_=pa[:, 0:8])
                    nc.sync.dma_start(
                        out=EX3[qw][hr0:hr0 + 128, 0:F3 + 8], in_=h3[:])
                if debug_stage >= layer + 2 and w == CHB[qw + 1] - 1:
                    ag(layer, qw)

            if debug_stage >= 2:
                edge_phase(1)
            if debug_stage >= 3:
                edge_phase(2)
            if debug_stage >= 4:
                edge_phase(3)
            if debug_stage < 4:
                zz = sb.tile([128, C3], DT_F32, tag="zz")
                nc.vector.memset(zz[:], 0.0)
                for _w in range(W):
                    nc.sync.dma_start(out=OUTI[_w * 128:(_w + 1) * 128, :],
                                      in_=zz[:])

            nc.sync.dma_start(out=OUT[:], in_=OUTI[0:SHARD, :])
            tk = sb.tile([128, 1], DT_F32, tag="tick")
            nc.sync.dma_start(out=tk[:], in_=P["tick"][:])
            nc.sync.dma_start(out=TOCK[:], in_=tk[:])

    _finalize(nc)
    return nc


def _finalize(nc):
    from concourse.bass import _bass_rust as _br
    from concourse.library_config import all_libraries, standard
    m = {}
    for lib in all_libraries:
        for it in lib.instructions:
            m[it] = m.get(it, 0) | (1 << lib.index)
    _br.insert_library_loads(nc, m, len(all_libraries), standard.index)
    mybir.codegen_inst_isa_subclasses(nc)
    _split_drain_waits(nc)


_CACHE = {}
_last_in_maps = None
_last_meta = None


def kernel(**inputs):
    global _last_in_maps, _last_meta
    x = np.asarray(inputs["x"], np.float32)
    edge_index = np.asarray(inputs["edge_index"], np.int32)
    meta, tables = host_prep(edge_index)
    use_bias = any(np.any(np.asarray(inputs[b]) != 0) for b in ("b1", "b2", "b3"))
    tables["use_bias"] = use_bias
    meta = meta + (use_bias,)
    if meta not in _CACHE:
        _CACHE[meta] = build_program(meta, tables)
    nc = _CACHE[meta]
    _last_meta = (meta, tables)

    W1 = np.asarray(inputs["W1"], np.float32)
    W2 = np.asarray(inputs["W2"], np.float32)
    W3 = np.asarray(inputs["W3"], np.float32)
    wa1 = np.concatenate([blockdiag(np.asarray(inputs["as1"], np.float32)),
                          blockdiag(np.asarray(inputs["ad1"], np.float32))], 1)
    wa2 = np.concatenate([blockdiag(np.asarray(inputs["as2"], np.float32)),
                          blockdiag(np.asarray(inputs["ad2"], np.float32))], 1)
    wa3 = np.concatenate([blockdiag(np.asarray(inputs["as3"], np.float32)),
                          blockdiag(np.asarray(inputs["ad3"], np.float32))], 1)
    iota = np.tile(np.arange(128, dtype=np.float32)[None, :], (128, 1))

    xT = np.ascontiguousarray(x.T)          # [12, N]
    xTfull = np.zeros((12, NC * SHARD_PAD), np.float32)
    for c in range(NC):
        xTfull[:, c * SHARD_PAD:c * SHARD_PAD + SHARD] = \
            xT[:, c * SHARD:(c + 1) * SHARD]

    com = {
        "xT": xTfull.astype(BF),
        "w1": W1.astype(BF),
        "wwa1": (W1 @ wa1).astype(BF),
        "w2c": W2.reshape(4, 128, F).astype(BF),
        "wwa2": (W2 @ wa2).reshape(4, 128, 16).astype(BF),
        "w3c": W3.reshape(4, 128, F3).astype(BF),
        "wwa3": (W3 @ wa3).reshape(4, 128, 16).astype(BF),
        "b1r": np.tile(np.asarray(inputs["b1"], np.float32)[None, :], (128, 1)),
        "b2r": np.tile(np.asarray(inputs["b2"], np.float32)[None, :], (128, 1)),
        "b3r": np.tile(np.asarray(inputs["b3"], np.float32)[None, :], (128, 1)),
        "iotab": iota.astype(BF),
        "identb": np.eye(128, dtype=np.float32).astype(BF),
        "tick": np.zeros((128, 1), np.float32),
    }
    ts = tables["ts"]
    in_maps = []
    for c in range(NC):
        m = dict(com)
        m["xTo"] = np.ascontiguousarray(
            xTfull[:, c * SHARD_PAD:(c + 1) * SHARD_PAD]).astype(BF)
        for q in range(NQ):
            m[f"idxQ{q}"] = ts["idxQ"][q][c]
        m["idxD"] = ts["idxD"][c]
        m["drow"] = ts["drow"][c]
        in_maps.append(m)
    _last_in_maps = in_maps
    res = run_bass_kernel_spmd(nc, in_maps, list(range(NC)))
    return np.concatenate([res.results[c]["out"] for c in range(NC)], axis=0)


# revision 33
# speedup vs baseline: 3.6117x; 3.6117x over previous
"""3-layer GAT (GATConv x3 + log_softmax) on 8 trn2 NeuronCores — v3.

Changes vs the v2 baseline:
- Layer-1 node phase is REPLICATED: every core computes h1|als1 for ALL
  nodes straight from the full x input (Fin=12, cheap on PE) and writes
  HF1 locally -> the 64MB layer-1 AllGather is gone entirely.  al_d for
  own windows comes from a tiny extra matmul against the own-shard
  input (xTo), since "own windows" is not static under SPMD.
- h rows are fp8(e4m3) 768B [512 fp8 h | 8 bf16 als | pad] instead of
  bf16 1280B: gather reads, HF writes and the boundary AllGathers all
  shrink 40%.  al_s rides in the row as raw bf16 bytes (bitcast view);
  al_d stays bf16 (256B rows, separate small gather).
- The h row space is split into 3 chunks (21/21/7 windows) with one
  Shared tensor per chunk, so each boundary AllGather fires as soon as
  its windows close (w=20/41/48) and overlaps the edge phase.  One
  unified id/table set serves all three layers.
- Node-phase HF1 stores are batched 7 windows per DMA (strided 3D out
  AP) - per-window DMAs serialized the SP queue.
- Engine balancing: PSUM->SBUF copies and exp-expansion alternate
  between ACT and DVE.
"""
import numpy as np
import ml_dtypes

import concourse.bass as bass
import concourse.mybir as mybir
import concourse.tile as tile
from concourse.bass_utils import run_bass_kernel_spmd

BF = ml_dtypes.bfloat16
N = 50000
NC = 8
SHARD = N // NC            # 6250
W = (SHARD + 127) // 128   # 49 windows of 128 dst nodes
SHARD_PAD = W * 128        # 6272
H, C = 8, 64
F = H * C                  # 512
C3 = 5
F3 = H * C3                # 40
EW = 640                   # (unused for h rows now; kept for reference)
EW8 = 768                  # fp8 h row width: 512 h fp8 | 16B als bf16 | pad
HB = 528                   # written bytes per h row
EW3 = 64                   # layer-3 row width (f32): 40 h | 8 als | pad
CHB = (0, 7, 28, 49)       # chunk window boundaries (AllGather chunks)
NQ = len(CHB) - 1
QNW = tuple(CHB[q + 1] - CHB[q] for q in range(NQ))    # (21, 21, 7)
QROWS = tuple(NC * nw * 128 for nw in QNW)             # per-chunk rows
NEG_SLOPE = 0.2
DT_BF = mybir.dt.bfloat16
DT_F32 = mybir.dt.float32
DT_I16 = mybir.dt.int16
DT_F8 = mybir.dt.float8e4
AF = mybir.ActivationFunctionType
ALU = mybir.AluOpType


PAIRS = [(2 * p, 2 * p + 1) for p in range(W // 2)] + [(W - 1,)]


def _split_drain_waits(nc, max_waits=1):
    # walrus on this toolchain rejects instructions carrying more than a few
    # sync waits; keep <=max_waits per instruction, move extras onto NoOps
    # inserted just before (same engine -> executes first, semantics kept).
    ctr = 0
    for f in nc.m.functions:
        for blk in f.blocks:
            new_list = []
            for ins in blk.instructions:
                if ins.sync_info and \
                        len(ins.sync_info.on_wait) > max_waits:
                    waits = list(ins.sync_info.on_wait)
                    keep, extra = waits[:max_waits], waits[max_waits:]
                    for w in extra:
                        ctr += 1
                        new_list.append(mybir.InstNoOp(
                            name=f"drainfix-{ctr}", engine=ins.engine,
                            ins=[], outs=[],
                            sync_info=mybir.SyncInfo(on_wait=[w], on_update=[])))
                    ins.sync_info.on_wait = keep
                new_list.append(ins)
            blk.instructions[:] = new_list


def _bcast(ap, ap_list):
    return bass.AP(ap.tensor, ap.offset, ap_list)


def blockdiag(a):
    Hh, cc = a.shape
    out = np.zeros((Hh * cc, Hh), np.float32)
    for h in range(Hh):
        out[h * cc:(h + 1) * cc, h] = a[h]
    return out


def _rowq(g):
    # unified chunked row space: (chunk q, row within chunk tensor)
    c, r = np.divmod(g, SHARD)
    w, d = np.divmod(r, 128)
    q = np.searchsorted(np.asarray(CHB), w, side="right") - 1
    base = np.asarray(CHB)[q]
    nw = np.asarray(QNW)[q]
    return q, c * nw * 128 + (w - base) * 128 + d


def _space_tables(src, dst):
    """Per-core gather tables for the unified chunked row space."""
    q_all, rows_all = _rowq(src)

    edges = [[[None] * W for _ in range(NC)] for _ in range(NQ)]
    for c in range(NC):
        lo, hi = c * SHARD, (c + 1) * SHARD
        m0 = np.searchsorted(dst, lo, "left")
        m1 = np.searchsorted(dst, hi, "left")
        d_c = dst[m0:m1] - lo
        rq = rows_all[m0:m1]
        qq = q_all[m0:m1]
        counts = np.bincount(d_c, minlength=SHARD)
        starts = np.concatenate([[0], np.cumsum(counts)])
        for w in range(W):
            n0, n1 = w * 128, min((w + 1) * 128, SHARD)
            e0, e1 = starts[n0], starts[n1]
            dw = d_c[e0:e1] - w * 128
            for q in range(NQ):
                mq = qq[e0:e1] == q
                edges[q][c][w] = (rq[e0:e1][mq].astype(np.int32), dw[mq])

    KQ = [[max(max(-(-len(edges[q][c][w][0]) // 128) for c in range(NC)), 1)
           for w in range(W)] for q in range(NQ)]

    pair_meta = []      # (ws, kq (NQ x len(ws)), colq (NQ), tile0)
    colq = [0] * NQ
    cT = 0
    for ws in PAIRS:
        kq = [[KQ[q][w] for w in ws] for q in range(NQ)]
        pair_meta.append((ws, kq, tuple(colq), cT))
        for q in range(NQ):
            colq[q] += sum(kq[q])
        cT += sum(sum(k) for k in kq)

    CQ = [colq[q] * 8 for q in range(NQ)]
    CT = cT

    def wrap16(flat, tbl, col0):
        a = flat.reshape(-1, 16)
        for g in range(8):
            tbl[16 * g:16 * g + 16, col0:col0 + a.shape[0]] = a.T

    idxQ = [np.zeros((NC, 128, CQ[q]), np.int16) for q in range(NQ)]
    idxD = np.zeros((NC, 128, CT * 8), np.int16)
    drow = np.full((NC, 128, CT), 999.0, np.float32)

    for c in range(NC):
        for (ws, kq, q0s, t0) in pair_meta:
            ntq = [sum(kq[q]) for q in range(NQ)]
            ktp = sum(ntq)
            flatD = np.zeros(ktp * 128, np.int16)
            flatR = np.full(ktp * 128, 999.0, np.float32)
            tb = 0
            for q in range(NQ):
                flatQ = np.zeros(ntq[q] * 128, np.int16)
                qb = 0
                for wi, w in enumerate(ws):
                    s, d = edges[q][c][w]
                    n = len(s)
                    flatQ[qb * 128:qb * 128 + n] = s.astype(np.int16)
                    flatD[(tb + qb) * 128:(tb + qb) * 128 + n] = \
                        (d + w * 128).astype(np.int16)
                    flatR[(tb + qb) * 128:(tb + qb) * 128 + n] = \
                        d.astype(np.float32)
                    qb += kq[q][wi]
                wrap16(flatQ, idxQ[q][c], q0s[q] * 8)
                tb += ntq[q]
            wrap16(flatD, idxD[c], t0 * 8)
            drow[c, :, t0:t0 + ktp] = flatR.reshape(-1, 128).T

    return dict(idxQ=idxQ, idxD=idxD, drow=drow,
                pair_meta=pair_meta, CQ=CQ, CT=CT,
                KQ=tuple(tuple(k) for k in KQ))


def host_prep(edge_index):
    src = np.concatenate([edge_index[0], np.arange(N, dtype=np.int32)])
    dst = np.concatenate([edge_index[1], np.arange(N, dtype=np.int32)])
    order = np.argsort(dst, kind="stable")
    src, dst = src[order], dst[order]
    ts = _space_tables(src, dst)
    meta = ts["KQ"]
    return meta, {"ts": ts}


def build_program(meta, tables, null=False, debug_stage=99):
    use_bias = tables.get("use_bias", True)
    ts = tables["ts"]

    nc = bass.Bass("TRN2")
    P = {}

    def par(name, shape, dt):
        P[name] = nc.declare_dram_parameter(name, list(shape), dt, isOutput=False)
        return P[name]

    par("xT", [12, NC * SHARD_PAD], DT_BF)
    par("xTo", [12, SHARD_PAD], DT_BF)
    par("w1", [12, F], DT_BF)
    par("wwa1", [12, 16], DT_BF)
    par("w2c", [4, 128, F], DT_BF)
    par("wwa2", [4, 128, 16], DT_BF)
    par("w3c", [4, 128, F3], DT_BF)
    par("wwa3", [4, 128, 16], DT_BF)
    par("b1r", [128, F], DT_F32)
    par("b2r", [128, F], DT_F32)
    par("b3r", [128, C3], DT_F32)
    par("iotab", [128, 128], DT_BF)
    par("identb", [128, 128], DT_BF)
    for q in range(NQ):
        par(f"idxQ{q}", [128, ts["CQ"][q]], DT_I16)
    par("idxD", [128, ts["CT"] * 8], DT_I16)
    par("drow", [128, ts["CT"]], DT_F32)
    par("tick", [128, 1], DT_F32)
    OUT = nc.declare_dram_parameter("out", [SHARD, C3], DT_F32, isOutput=True)
    TOCK = nc.declare_dram_parameter("tock", [128, 1], DT_F32, isOutput=True)

    if null:
        with tile.TileContext(nc) as tc:
            with tc.tile_pool(name="s", bufs=1) as s0:
                z = s0.tile([128, C3], DT_F32)
                nc.vector.memset(z[:], 0.0)
                for r0 in range(0, SHARD, 128):
                    nc.sync.dma_start(out=OUT[r0:min(r0 + 128, SHARD), :],
                                      in_=z[:min(128, SHARD - r0), :])
                tk = s0.tile([128, 1], DT_F32)
                nc.sync.dma_start(out=tk[:], in_=P["tick"][:])
                nc.sync.dma_start(out=TOCK[:], in_=tk[:])
        _finalize(nc)
        return nc

    rg = [list(range(NC))]
    with tile.TileContext(nc) as tc:
        with (
            tc.tile_pool(name="const", bufs=1) as cp,
            tc.tile_pool(name="sbuf", bufs=3) as sb,
            tc.tile_pool(name="stage", bufs=2) as stg,
            tc.tile_pool(name="selp", bufs=8) as selp,
            tc.tile_pool(name="xstg", bufs=1) as xstg,
            tc.tile_pool(name="gath", bufs=3) as gth,
            tc.tile_pool(name="psU", bufs=3, space="PSUM") as p_U,
            tc.tile_pool(name="psX", bufs=2, space="PSUM") as p_X,
            tc.tile_pool(name="psT", bufs=2, space="PSUM") as p_T,
            tc.tile_pool(name="dram", bufs=1, space="DRAM") as dr,
        ):
            # ---------------- constants ----------------
            t_iota = cp.tile([128, 128], DT_BF)
            nc.sync.dma_start(out=t_iota[:], in_=P["iotab"][:])
            t_ident = cp.tile([128, 128], DT_BF)
            nc.sync.dma_start(out=t_ident[:], in_=P["identb"][:])
            t_xTo = cp.tile([12, SHARD_PAD], DT_BF)
            nc.sync.dma_start(out=t_xTo[:], in_=P["xTo"][:])
            t_w1 = cp.tile([12, F], DT_BF)
            nc.sync.dma_start(out=t_w1[:], in_=P["w1"][:])
            t_wwa1 = cp.tile([12, 16], DT_BF)
            nc.sync.dma_start(out=t_wwa1[:], in_=P["wwa1"][:])
            t_w2 = cp.tile([128, 4, F], DT_BF)
            t_wwa2 = cp.tile([128, 4, 16], DT_BF)
            t_w3 = cp.tile([128, 4, F3], DT_BF)
            t_wwa3 = cp.tile([128, 4, 16], DT_BF)
            for ch in range(4):
                nc.sync.dma_start(out=t_w2[:, ch, :], in_=P["w2c"][ch])
                nc.sync.dma_start(out=t_wwa2[:, ch, :], in_=P["wwa2"][ch])
                nc.sync.dma_start(out=t_w3[:, ch, :], in_=P["w3c"][ch])
                nc.sync.dma_start(out=t_wwa3[:, ch, :], in_=P["wwa3"][ch])
            if use_bias:
                t_b1 = cp.tile([128, F], DT_F32)
                nc.sync.dma_start(out=t_b1[:], in_=P["b1r"][:])
                t_b2 = cp.tile([128, F], DT_F32)
                nc.sync.dma_start(out=t_b2[:], in_=P["b2r"][:])
            t_b3 = cp.tile([128, C3], DT_F32)
            if use_bias:
                nc.sync.dma_start(out=t_b3[:], in_=P["b3r"][:])
            t_idxQ = []
            for q in range(NQ):
                tq = cp.tile([128, ts["CQ"][q]], DT_I16, name=f"t_idxQ{q}")
                nc.sync.dma_start(out=tq[:], in_=P[f"idxQ{q}"][:])
                t_idxQ.append(tq)
            t_idxD = cp.tile([128, ts["CT"] * 8], DT_I16)
            nc.sync.dma_start(out=t_idxD[:], in_=P["idxD"][:])
            t_drow = cp.tile([128, ts["CT"]], DT_F32)
            nc.sync.dma_start(out=t_drow[:], in_=P["drow"][:])

            # ---------------- DRAM internals ----------------
            HF1 = [dr.tile([QROWS[q], EW8], DT_F8, name=f"HF1_{q}")
                   for q in range(NQ)]
            AD1 = dr.tile([SHARD_PAD, 128], DT_BF)
            AD2 = dr.tile([SHARD_PAD, 128], DT_BF)
            AD3 = dr.tile([SHARD_PAD, EW3], DT_F32)
            EX2 = [dr.tile([QNW[q] * 128, EW8], DT_F8, name=f"EX2_{q}")
                   for q in range(NQ)]
            HF2 = [dr.tile([QROWS[q], EW8], DT_F8, addr_space="Shared",
                           name=f"HF2_{q}") for q in range(NQ)]
            EX3 = [dr.tile([QNW[q] * 128, EW3], DT_F32, name=f"EX3_{q}")
                   for q in range(NQ)]
            H3F = [dr.tile([QROWS[q], EW3], DT_F32, addr_space="Shared",
                           name=f"H3F_{q}") for q in range(NQ)]
            OUTI = dr.tile([SHARD_PAD, C3], DT_F32)

            # ---------------- layer-1 node phase (replicated) ------------
            # al_d for own windows, from own-shard input
            for w in range(W):
                pao = p_X.tile([128, 24], DT_F32, space="PSUM", tag="paux")
                nc.tensor.matmul(pao[:, 0:8], lhsT=t_xTo[:, w * 128:(w + 1) * 128],
                                 rhs=t_wwa1[:, 8:16], start=True, stop=True)
                ast = stg.tile([128, 8], DT_BF, tag="ast")
                nc.vector.tensor_copy(out=ast[:], in_=pao[:, 0:8])
                nc.sync.dma_start(out=AD1[w * 128:(w + 1) * 128, 0:8],
                                  in_=ast[:])
            # h1 | als1 for ALL nodes; batched 7-window DMA writes
            for blk in range(NC):
                xch = xstg.tile([12, SHARD_PAD], DT_BF, tag="xch")
                nc.sync.dma_start(
                    out=xch[:],
                    in_=P["xT"][:, blk * SHARD_PAD:(blk + 1) * SHARD_PAD])
                for w7 in range(W // 7):
                    hst = stg.tile([128, 7, HB], DT_F8, tag="hst")
                    for wj in range(7):
                        w = w7 * 7 + wj
                        lhs = xch[:, w * 128:(w + 1) * 128]
                        ph = p_U.tile([128, F], DT_F32, space="PSUM",
                                      tag="pout")
                        nc.tensor.matmul(ph[:], lhsT=lhs, rhs=t_w1[:],
                                         start=True, stop=True)
                        pa = p_X.tile([128, 24], DT_F32, space="PSUM",
                                      tag="paux")
                        nc.tensor.matmul(pa[:, 0:8], lhsT=lhs,
                                         rhs=t_wwa1[:, 0:8],
                                         start=True, stop=True)
                        if (blk + w) % 2 == 0:
                            nc.scalar.activation(hst[:, wj, 0:F], ph[:],
                                                 AF.Copy)
                        else:
                            nc.vector.tensor_copy(out=hst[:, wj, 0:F],
                                                  in_=ph[:])
                        nc.vector.tensor_copy(
                            out=hst[:, wj, F:F + 16].bitcast(DT_BF),
                            in_=pa[:, 0:8])
                    w0 = w7 * 7
                    q = next(i for i in range(NQ) if w0 < CHB[i + 1])
                    hf = HF1[q]
                    r0 = blk * QNW[q] * 128 + (w0 - CHB[q]) * 128
                    out_ap = bass.AP(
                        hf[:].tensor, (r0 * EW8) + hf[:].offset,
                        [[EW8, 128], [128 * EW8, 7], [1, HB]])
                    nc.sync.dma_start(out=out_ap, in_=hst[:])

            # ---------------- edge phases ----------------
            _regs = {}

            def nreg(v):
                if v not in _regs:
                    _regs[v] = nc.gpsimd.to_reg(v)
                return _regs[v]

            def edge_phase(layer):
                lay3 = layer == 3
                ADt = (AD1, AD2, AD3)[layer - 1]
                ald_ew = EW3 if lay3 else 128
                srcQ = (HF1, HF2, H3F)[layer - 1]
                ew = EW3 if lay3 else EW8
                hdt = DT_F32 if lay3 else DT_F8
                fh = F3 if lay3 else F
                cw = C3 if lay3 else C
                gdt = DT_F32 if lay3 else DT_BF
                als_off = F3 if lay3 else F
                for pi, (ws, kq, q0s, t0) in enumerate(ts["pair_meta"]):
                    ntq = [sum(kq[q]) for q in range(NQ)]
                    ktp = sum(ntq)
                    hg = gth.tile([128, ktp, ew], hdt, tag="hg")
                    tb = 0
                    for q in range(NQ):
                        nc.gpsimd.dma_gather(
                            hg[:, tb:tb + ntq[q], :], srcQ[q][:],
                            t_idxQ[q][:, q0s[q] * 8:(q0s[q] + ntq[q]) * 8],
                            ntq[q] * 128, nreg(ntq[q] * 128), ew,
                            single_packet=False)
                        tb += ntq[q]
                    ald = gth.tile([128, ktp, ald_ew], gdt, tag="ald")
                    nc.gpsimd.dma_gather(
                        ald[:], ADt[:], t_idxD[:, t0 * 8:(t0 + ktp) * 8],
                        ktp * 128, nreg(ktp * 128), ald_ew,
                        single_packet=False)
                    sels = []
                    for wi, w in enumerate(ws):
                        regions = []
                        tb = 0
                        for q in range(NQ):
                            regions.append((tb + sum(kq[q][:wi]), kq[q][wi]))
                            tb += ntq[q]
                        rsels = []
                        for (rb, rl) in regions:
                            sel = selp.tile([128, rl, 128], DT_BF, tag="sel")
                            for j in range(rl):
                                nc.vector.tensor_scalar(
                                    out=sel[:, j, :], in0=t_iota[:],
                                    scalar1=t_drow[:, t0 + rb + j:t0 + rb + j + 1],
                                    scalar2=None, op0=ALU.is_equal)
                            rsels.append((rb, rl, sel))
                        sels.append(rsels)
                    # e = als[src] + ald[dst]; ex = exp(lrelu(e))
                    ald_off = F3 + 8 if lay3 else 0
                    e_t = sb.tile([128, ktp, 8], gdt, tag="e")
                    if lay3:
                        als_v = hg[:, :, als_off:als_off + 8]
                    else:
                        als_v = hg[:, :, F:F + 16].bitcast(DT_BF)
                    nc.vector.tensor_tensor(
                        out=e_t[:], in0=als_v,
                        in1=ald[:, :, ald_off:ald_off + 8], op=ALU.add)
                    lr = sb.tile([128, ktp, 8], gdt, tag="lr")
                    nc.vector.scalar_tensor_tensor(
                        out=lr[:], in0=e_t[:], scalar=NEG_SLOPE,
                        in1=e_t[:], op0=ALU.mult, op1=ALU.max)
                    exb = sb.tile([128, ktp, 8], DT_BF, tag="exb")
                    nc.scalar.activation(exb[:], lr[:], AF.Exp)
                    # scatter per window
                    for wi, w in enumerate(ws):
                        poutF = p_U.tile([128, F], DT_F32, space="PSUM",
                                         tag="pout")
                        pout = poutF[:, 0:fh]
                        paux = p_X.tile([128, 24], DT_F32, space="PSUM",
                                        tag="paux")
                        pden = paux[:, 0:8]
                        nt_w = sum(kq[q][wi] for q in range(NQ))
                        i = 0
                        for (rb, rl, sel) in sels[wi]:
                            exb_r = exb[:, rb:rb + rl, :]
                            exb_b = _bcast(exb_r, [exb_r.ap[0], [8, rl],
                                                   [1, 8], [0, cw]])
                            msg = sb.tile([128, rl, fh], DT_BF, tag="msg")
                            msg_4 = _bcast(msg[:], [msg[:].ap[0], [fh, rl],
                                                    [cw, 8], [1, cw]])
                            hg_r = hg[:, rb:rb + rl, 0:fh]
                            hg_4 = _bcast(hg_r, [hg_r.ap[0], [ew, rl],
                                                 [cw, 8], [1, cw]])
                            nc.vector.tensor_tensor(
                                out=msg_4, in0=hg_4, in1=exb_b, op=ALU.mult)
                            for j in range(rl):
                                st, sp_ = i == 0, i == nt_w - 1
                                nc.tensor.matmul(pout[:], lhsT=sel[:, j, :],
                                                 rhs=msg[:, j, :],
                                                 start=st, stop=sp_)
                                nc.tensor.matmul(pden[:], lhsT=sel[:, j, :],
                                                 rhs=exb[:, rb + j, :],
                                                 start=st, stop=sp_)
                                i += 1
                        _close(layer, w, pout, pden, paux)

            def ag(layer, q):
                exh = EX2[q] if layer == 1 else EX3[q]
                hfh = HF2[q] if layer == 1 else H3F[q]
                nc.gpsimd.collective_compute(
                    "AllGather", ALU.bypass, replica_groups=rg,
                    ins=[exh[:].opt()], outs=[hfh[:].opt()])

            def _close(layer, w, pout, pden, paux):
                lay3 = layer == 3
                fh = F3 if lay3 else F
                cw = C3 if lay3 else C
                r0 = w * 128
                den = sb.tile([128, 8], DT_F32, tag="den")
                nc.vector.tensor_scalar_add(den[:], pden[:], 1e-16)
                rec = sb.tile([128, 8], DT_F32, tag="rec")
                nc.vector.reciprocal(rec[:], den[:])
                onrm = sb.tile([128, fh], DT_F32, tag="onrm")
                rec_b = _bcast(rec[:], [rec[:].ap[0], [1, 8], [0, cw]])
                po4 = _bcast(pout[:], [pout[:].ap[0], [cw, 8], [1, cw]])
                on4 = _bcast(onrm[:], [onrm[:].ap[0], [cw, 8], [1, cw]])
                nc.vector.tensor_tensor(out=on4, in0=po4, in1=rec_b, op=ALU.mult)
                if lay3:
                    hm = sb.tile([128, C3], DT_F32, tag="hm")
                    on_T = _bcast(onrm[:], [onrm[:].ap[0], [1, C3], [C3, 8]])
                    nc.vector.reduce_sum(hm[:], on_T, axis=mybir.AxisListType.X)
                    nc.vector.tensor_scalar_mul(hm[:], hm[:], 0.125)
                    if use_bias:
                        nc.vector.tensor_add(out=hm[:], in0=hm[:], in1=t_b3[:])
                    mx = sb.tile([128, 1], DT_F32, tag="mx")
                    nc.vector.reduce_max(mx[:], hm[:], axis=mybir.AxisListType.X)
                    xc = sb.tile([128, C3], DT_F32, tag="xc")
                    nc.vector.tensor_tensor(out=xc[:], in0=hm[:],
                                            in1=mx[:].to_broadcast([128, C3]),
                                            op=ALU.subtract)
                    e5 = sb.tile([128, C3], DT_F32, tag="e5")
                    nc.scalar.activation(e5[:], xc[:], AF.Exp)
                    s5 = sb.tile([128, 1], DT_F32, tag="s5")
                    nc.vector.reduce_sum(s5[:], e5[:], axis=mybir.AxisListType.X)
                    lg = sb.tile([128, 1], DT_F32, tag="lg")
                    nc.scalar.activation(lg[:], s5[:], AF.Ln)
                    res = sb.tile([128, C3], DT_F32, tag="res")
                    nc.vector.tensor_tensor(out=res[:], in0=xc[:],
                                            in1=lg[:].to_broadcast([128, C3]),
                                            op=ALU.subtract)
                    nc.sync.dma_start(out=OUTI[r0:r0 + 128, :], in_=res[:])
                    return
                if use_bias:
                    xb = sb.tile([128, F], DT_F32, tag="xb")
                    nc.vector.tensor_add(out=xb[:], in0=onrm[:],
                                         in1=t_b1[:] if layer == 1 else t_b2[:])
                else:
                    xb = onrm
                xn = sb.tile([128, F], DT_BF, tag="xn")
                nc.scalar.activation(xn[:], xb[:], AF.Relu)
                xnT = sb.tile([128, 4, 128], DT_BF, tag="xnT")
                for ch in range(4):
                    ptx = p_T.tile([128, 128], DT_BF, space="PSUM", tag="ptxT")
                    nc.tensor.transpose(ptx[:], xn[:, ch * 128:(ch + 1) * 128],
                                        t_ident[:])
                    nc.scalar.activation(xnT[:, ch, :], ptx[:], AF.Copy)
                wN = t_w2 if layer == 1 else t_w3
                wwaN = t_wwa2 if layer == 1 else t_wwa3
                fn = F if layer == 1 else F3
                phF = p_U.tile([128, F], DT_F32, space="PSUM", tag="pout")
                ph = phF[:, 0:fn]
                pa = paux[:, 8:24]
                for ch in range(4):
                    nc.tensor.matmul(ph[:], lhsT=xnT[:, ch, :], rhs=wN[:, ch, :],
                                     start=(ch == 0), stop=(ch == 3))
                    nc.tensor.matmul(pa[:], lhsT=xnT[:, ch, :], rhs=wwaN[:, ch, :],
                                     start=(ch == 0), stop=(ch == 3))
                qw = next(i for i in range(NQ) if w < CHB[i + 1])
                hr0 = (w - CHB[qw]) * 128
                r0w = w * 128
                if layer == 1:
                    ast = stg.tile([128, 8], DT_BF, tag="ast")
                    nc.vector.tensor_copy(out=ast[:], in_=pa[:, 8:16])
                    nc.sync.dma_start(out=AD2[r0w:r0w + 128, 0:8], in_=ast[:])
                else:
                    as3 = stg.tile([128, 8], DT_F32, tag="as3")
                    nc.vector.tensor_copy(out=as3[:], in_=pa[:, 8:16])
                    nc.sync.dma_start(
                        out=AD3[r0w:r0w + 128, F3 + 8:F3 + 16], in_=as3[:])
                if layer == 1:
                    hst = stg.tile([128, HB], DT_F8, tag="hstc")
                    nc.scalar.activation(hst[:, 0:F], ph[:], AF.Copy)
                    nc.vector.tensor_copy(
                        out=hst[:, F:F + 16].bitcast(DT_BF), in_=pa[:, 0:8])
                    nc.sync.dma_start(
                        out=EX2[qw][hr0:hr0 + 128, 0:HB], in_=hst[:])
                else:
                    h3 = stg.tile([128, F3 + 8], DT_F32, tag="h3")
                    nc.vector.tensor_copy(out=h3[:, 0:F3], in_=ph[:])
                    nc.vector.tensor_copy(out=h3[:, F3:F3 + 8], in_=pa[:, 0:8])
                    nc.sync.dma_start(
                        out=EX3[qw][hr0:hr0 + 128, 0:F3 + 8], in_=h3[:])
                if debug_stage >= layer + 2 and w == CHB[qw + 1] - 1:
                    ag(layer, qw)

            if debug_stage >= 2:
                edge_phase(1)
            if debug_stage >= 3:
                edge_phase(2)
            if debug_stage >= 4:
                edge_phase(3)
            if debug_stage < 4:
                zz = sb.tile([128, C3], DT_F32, tag="zz")
                nc.vector.memset(zz[:], 0.0)
                for _w in range(W):
                    nc.sync.dma_start(out=OUTI[_w * 128:(_w + 1) * 128, :],
                                      in_=zz[:])

            nc.sync.dma_start(out=OUT[:], in_=OUTI[0:SHARD, :])
            tk = sb.tile([128, 1], DT_F32, tag="tick")
            nc.sync.dma_start(out=tk[:], in_=P["tick"][:])
            nc.sync.dma_start(out=TOCK[:], in_=tk[:])

    _finalize(nc)
    return nc


def _finalize(nc):
    from concourse.bass import _bass_rust as _br
    from concourse.library_config import all_libraries, standard
    m = {}
    for lib in all_libraries:
        for it in lib.instructions:
            m[it] = m.get(it, 0) | (1 << lib.index)
    _br.insert_library_loads(nc, m, len(all_libraries), standard.index)
    mybir.codegen_inst_isa_subclasses(nc)
    _split_drain_waits(nc)


_CACHE = {}
_last_in_maps = None
_last_meta = None


def kernel(**inputs):
    global _last_in_maps, _last_meta
    x = np.asarray(inputs["x"], np.float32)
    edge_index = np.asarray(inputs["edge_index"], np.int32)
    meta, tables = host_prep(edge_index)
    use_bias = any(np.any(np.asarray(inputs[b]) != 0) for b in ("b1", "b2", "b3"))
    tables["use_bias"] = use_bias
    meta = meta + (use_bias,)
    if meta not in _CACHE:
        _CACHE[meta] = build_program(meta, tables)
    nc = _CACHE[meta]
    _last_meta = (meta, tables)

    W1 = np.asarray(inputs["W1"], np.float32)
    W2 = np.asarray(inputs["W2"], np.float32)
    W3 = np.asarray(inputs["W3"], np.float32)
    wa1 = np.concatenate([blockdiag(np.asarray(inputs["as1"], np.float32)),
                          blockdiag(np.asarray(inputs["ad1"], np.float32))], 1)
    wa2 = np.concatenate([blockdiag(np.asarray(inputs["as2"], np.float32)),
                          blockdiag(np.asarray(inputs["ad2"], np.float32))], 1)
    wa3 = np.concatenate([blockdiag(np.asarray(inputs["as3"], np.float32)),
                          blockdiag(np.asarray(inputs["ad3"], np.float32))], 1)
    iota = np.tile(np.arange(128, dtype=np.float32)[None, :], (128, 1))

    xT = np.ascontiguousarray(x.T)          # [12, N]
    xTfull = np.zeros((12, NC * SHARD_PAD), np.float32)
    for c in range(NC):
        xTfull[:, c * SHARD_PAD:c * SHARD_PAD + SHARD] = \
            xT[:, c * SHARD:(c + 1) * SHARD]

    com = {
        "xT": xTfull.astype(BF),
        "w1": W1.astype(BF),
        "wwa1": (W1 @ wa1).astype(BF),
        "w2c": W2.reshape(4, 128, F).astype(BF),
        "wwa2": (W2 @ wa2).reshape(4, 128, 16).astype(BF),
        "w3c": W3.reshape(4, 128, F3).astype(BF),
        "wwa3": (W3 @ wa3).reshape(4, 128, 16).astype(BF),
        "b1r": np.tile(np.asarray(inputs["b1"], np.float32)[None, :], (128, 1)),
        "b2r": np.tile(np.asarray(inputs["b2"], np.float32)[None, :], (128, 1)),
        "b3r": np.tile(np.asarray(inputs["b3"], np.float32)[None, :], (128, 1)),
        "iotab": iota.astype(BF),
        "identb": np.eye(128, dtype=np.float32).astype(BF),
        "tick": np.zeros((128, 1), np.float32),
    }
    ts = tables["ts"]
    in_maps = []
    for c in range(NC):
        m = dict(com)
        m["xTo"] = np.ascontiguousarray(
            xTfull[:, c * SHARD_PAD:(c + 1) * SHARD_PAD]).astype(BF)
        for q in range(NQ):
            m[f"idxQ{q}"] = ts["idxQ"][q][c]
        m["idxD"] = ts["idxD"][c]
        m["drow"] = ts["drow"][c]
        in_maps.append(m)
    _last_in_maps = in_maps
    res = run_bass_kernel_spmd(nc, in_maps, list(range(NC)))
    return np.concatenate([res.results[c]["out"] for c in range(NC)], axis=0)
